# revision 6
# baseline (speedup 1.0000x reference)
"""MoE (top-2 of 8 experts) Trainium2 kernel, expert-parallel across 8 NeuronCores.

Sharding: expert-parallel. Core e holds expert e's weights and receives the
tokens routed to it (host-side all-to-all by routing decision, per the
sharding hint). On device, each core computes the routing logits for its
tokens, the top-2 softmax gate for its own expert (the selection is encoded in
a host-provided +-1 one-hot difference vector, so the device never needs an
argmax; gate = sigmoid(l_own - l_other)), the gated expert matmul (fp32r on
the PE at full rate), and its expert's importance sum for the aux loss.

Host side does only sharding/unsharding: gather/scatter-add of token rows
(each token's output is the sum of its two expert contributions) and the
final 8-element cv^2 reduction for the loss scalar.
"""

import numpy as np

E = 8
D = 1024
H = 4096
LOSS_COEF = 0.01
EPS = 1e-10
NK = D // 128  # contraction chunks
NH = H // 512  # output column chunks


def _chunks(total, size):
    out, s = [], 0
    while s < total:
        out.append((s, min(size, total - s)))
        s += size
    return out


def _build(t_pad: int, with_bias: bool):
    from concourse import bacc, mybir
    import concourse.tile as tile

    mch = t_pad // 128
    tch = _chunks(t_pad, 384)  # token chunks for the gate pipeline
    f32 = mybir.dt.float32
    f32r = mybir.dt.float32r
    AF = mybir.ActivationFunctionType
    mult = mybir.AluOpType.mult

    nc = bacc.Bacc(None)
    xt_d = nc.declare_dram_parameter("xt", [D, t_pad], f32r, isOutput=False)
    w_d = nc.declare_dram_parameter("w", [D, H], f32r, isOutput=False)
    wg_d = nc.declare_dram_parameter("wg", [D, E], f32r, isOutput=False)
    sel_d = nc.declare_dram_parameter("sel", [E, t_pad], f32, isOutput=False)
    pm_d = nc.declare_dram_parameter("pm", [1, t_pad], f32, isOutput=False)
    if with_bias:
        bb_d = nc.declare_dram_parameter("bb", [128, H], f32, isOutput=False)
    u_d = nc.declare_dram_parameter("u", [t_pad, H], f32, isOutput=True)
    imp_d = nc.declare_dram_parameter("imp", [1, 1], f32, isOutput=True)

    with tile.TileContext(nc) as tc:
        with (
            tc.tile_pool(name="res", bufs=1) as res_pool,
            tc.tile_pool(name="gat", bufs=1) as gat_pool,
            tc.tile_pool(name="wts", bufs=3 * NK) as w_pool,
            tc.tile_pool(name="out", bufs=6) as out_pool,
            tc.tile_pool(name="psm", bufs=5, space="PSUM") as psm,
            tc.tile_pool(name="psl", bufs=2, space="PSUM") as psl,
            tc.tile_pool(name="psd", bufs=1, space="PSUM") as psd,
            tc.tile_pool(name="dscr", bufs=1, space="DRAM") as dscr,
        ):
            # ---- static SBUF residents; xt split per (k, token-group) so
            # consumers unblock as each slab lands ----
            xt_sb = {}
            for g, (t0, tn) in enumerate(tch):
                for k in range(NK):
                    t = res_pool.tile([128, tn], f32r, tag=f"xt{k}_{g}")
                    nc.sync.dma_start(
                        t[:], xt_d[k * 128 : (k + 1) * 128, t0 : t0 + tn]
                    )
                    xt_sb[k, g] = t

            def xt_col(k, m):
                # [128, 128] lhsT slice for token m-chunk
                g, off = divmod(m * 128, 384)
                return xt_sb[k, g][:, off : off + 128]

            wg_sb = res_pool.tile([128, NK, E], f32r, tag="wg")
            for k in range(NK):
                nc.sync.dma_start(wg_sb[:, k, :], wg_d[k * 128 : (k + 1) * 128, :])
            sel_sb = res_pool.tile([E, t_pad], f32, tag="sel")
            nc.sync.dma_start(sel_sb[:], sel_d[:])
            pm_sb = res_pool.tile([1, t_pad], f32, tag="pm")
            nc.sync.dma_start(pm_sb[:], pm_d[:])
            ones8 = res_pool.tile([E, 1], f32, tag="ones8")
            nc.vector.memset(ones8[:], 1.0)
            if with_bias:
                bb_sb = res_pool.tile([128, H], f32, tag="bb")
                nc.sync.dma_start(bb_sb[:], bb_d[:])

            # W tiles, one per (n, k); pool keeps 3 n-blocks in flight
            def load_w_block(n):
                tiles = []
                for k in range(NK):
                    t = w_pool.tile([128, 512], f32r, tag="w")
                    nc.sync.dma_start(
                        t[:], w_d[k * 128 : (k + 1) * 128, n * 512 : (n + 1) * 512]
                    )
                    tiles.append(t)
                return tiles

            gcol = gat_pool.tile([128, mch], f32)  # per-token gate, token-major
            g_row = gat_pool.tile([1, t_pad], f32, tag="grow")

            wt0 = load_w_block(0)

            def emit_mms(wt, m):
                ps = psm.tile([128, 512], f32)
                for k in range(NK):
                    nc.tensor.matmul(
                        ps[:],
                        xt_col(k, m),
                        wt[k][:],
                        start=(k == 0),
                        stop=(k == NK - 1),
                    )
                return ps

            def emit_drain(ps, n, m):
                ot = out_pool.tile([128, 512], f32)
                nc.scalar.activation(ot[:], ps[:], AF.Copy, scale=gcol[:, m : m + 1])
                if with_bias:
                    nc.vector.scalar_tensor_tensor(
                        ot[:],
                        bb_sb[:, n * 512 : (n + 1) * 512],
                        gcol[:, m : m + 1],
                        ot[:],
                        op0=mult,
                        op1=mybir.AluOpType.add,
                    )
                nc.sync.dma_start(
                    u_d[m * 128 : (m + 1) * 128, n * 512 : (n + 1) * 512], ot[:]
                )

            def main_group(wt, n, ms):
                for m in ms:
                    emit_drain(emit_mms(wt, m), n, m)

            # ---- gates, expert-major; interleaved with the first n-block's
            # matmuls (held in PSUM, drained after the gates land) so the PE
            # has work while xt streams in ----
            held = []  # (psum tile, m) for n=0, waiting on gcol
            for g, (t0, tn) in enumerate(tch):
                ms = [m for m in range(mch) if t0 <= m * 128 < t0 + tn]
                for m in ms:
                    if len(held) < 5:
                        held.append((emit_mms(wt0, m), m))
                # logitsT chunk [E, tn] = sum_k wg[k].T @ xt[k, chunk]
                lps = psl.tile([8, 512], f32)
                for k in range(NK):
                    nc.tensor.matmul(
                        lps[:E, :tn],
                        wg_sb[:, k, :],
                        xt_sb[k, g][:],
                        start=(k == 0),
                        stop=(k == NK - 1),
                    )
                # ld chunk [1, tn] = ones8.T @ (logitsT * selT)
                prod = gat_pool.tile([E, 512], f32, tag="prod")
                nc.vector.tensor_mul(
                    prod[:E, :tn], lps[:E, :tn], sel_sb[:, t0 : t0 + tn]
                )
                dps = psd.tile([1, 512], f32)
                nc.tensor.matmul(
                    dps[:1, :tn], ones8[:], prod[:E, :tn], start=True, stop=True
                )
                # g_row chunk = sigmoid(ld) * padmask
                sig = gat_pool.tile([1, 512], f32, tag="sig")
                nc.scalar.activation(sig[:1, :tn], dps[:1, :tn], AF.Sigmoid)
                nc.vector.tensor_mul(
                    g_row[:, t0 : t0 + tn], sig[:1, :tn], pm_sb[:, t0 : t0 + tn]
                )

            # importance = sum(g_row); token-major gate columns via tiny
            # SBUF->SBUF transpose DMA
            imp_sb = gat_pool.tile([1, 1], f32, tag="imp")
            nc.vector.reduce_sum(imp_sb[:], g_row[:], axis=mybir.AxisListType.X)
            nc.sync.dma_start(imp_d[:], imp_sb[:])
            # token-major gate columns: bounce through DRAM (an SBUF AP can't
            # turn a free index into a partition index; DRAM is flat so the
            # rearrange is legal there)
            gscr = dscr.tile([1, t_pad], f32)
            nc.sync.dma_start(gscr[:], g_row[:])
            nc.sync.dma_start(
                gcol[:], gscr.rearrange("a (m p) -> p (m a)", p=128)
            )

            # drain the held n=0 tiles, then the rest of n=0
            held_ms = [m for _, m in held]
            for ps, m in held:
                emit_drain(ps, 0, m)
            main_group(wt0, 0, [m for m in range(mch) if m not in held_ms])

            # ---- remaining n-blocks ----
            for n in range(1, NH):
                wt = load_w_block(n)
                main_group(wt, n, range(mch))
    nc.compile()
    return nc


def _cv_squared(v: np.ndarray) -> np.float32:
    v = v.astype(np.float32)
    return np.float32(v.var(ddof=1) / (v.mean() ** 2 + EPS))


def kernel(x, w_gate, expert_w, expert_b):
    from concourse.bass_utils import run_bass_kernel_spmd

    x = np.asarray(x, dtype=np.float32)
    w_gate = np.asarray(w_gate, dtype=np.float32)
    expert_w = np.ascontiguousarray(np.asarray(expert_w, dtype=np.float32))
    expert_b = np.asarray(expert_b, dtype=np.float32)
    B, S, _ = x.shape
    N = B * S
    xf = x.reshape(N, D)

    # ---- host routing (sharding decision only; fp64 so the top-2 selection
    # matches the fp32 reference even through near-ties) ----
    logits = xf.astype(np.float64) @ w_gate.astype(np.float64)
    order = np.argsort(-logits, axis=1, kind="stable")
    top1, top2 = order[:, 0].copy(), order[:, 1].copy()

    idx = [np.nonzero((top1 == e) | (top2 == e))[0] for e in range(E)]
    counts = np.array([len(i) for i in idx], dtype=np.int64)
    t_pad = max(384, int(-(-counts.max() // 128) * 128))

    with_bias = bool(np.any(expert_b))
    nc = _build(t_pad, with_bias)

    in_maps = []
    for e in range(E):
        ids = idx[e]
        t = len(ids)
        xt = np.zeros((D, t_pad), np.float32)
        xt[:, :t] = xf[ids].T
        sel = np.zeros((E, t_pad), np.float32)
        rows = np.arange(t)
        sel[e, rows] = 1.0
        other = np.where(top1[ids] == e, top2[ids], top1[ids])
        sel[other, rows] -= 1.0
        pm = np.zeros((1, t_pad), np.float32)
        pm[0, :t] = 1.0
        m = {"xt": xt, "w": expert_w[e], "wg": w_gate, "sel": sel, "pm": pm}
        if with_bias:
            m["bb"] = np.broadcast_to(expert_b[e], (128, H)).copy()
        in_maps.append(m)

    res = run_bass_kernel_spmd(nc, in_maps, list(range(E)))
    kernel.last_results = res

    # ---- unshard: scatter-add the two expert contributions per token ----
    y = np.zeros((N, H), np.float32)
    imp = np.zeros(E, np.float32)
    for e in range(E):
        u = res.results[e]["u"]
        y[idx[e]] += u[: counts[e]]
        imp[e] = res.results[e]["imp"][0, 0]
    load = counts.astype(np.float32)
    loss = np.float32(LOSS_COEF) * (_cv_squared(imp) + _cv_squared(load))
    return y.reshape(B, S, H), np.float32(loss)


# revision 8
# speedup vs baseline: 1.0339x; 1.0339x over previous
"""MoE (top-2 of 8 experts) Trainium2 kernel, expert-parallel across 8 NeuronCores.

Sharding: expert-parallel. Core e holds expert e's weights and receives the
tokens routed to it (host-side all-to-all by routing decision, per the
sharding hint). On device, each core computes the routing logits for its
tokens, the top-2 softmax gate for its own expert (the selection is encoded in
a host-provided +-1 one-hot difference vector, so the device never needs an
argmax; gate = sigmoid(l_own - l_other)), the gated expert matmul (fp32r on
the PE at full rate), and its expert's importance sum for the aux loss.

Host side does only sharding/unsharding: gather/scatter-add of token rows
(each token's output is the sum of its two expert contributions) and the
final 8-element cv^2 reduction for the loss scalar.
"""

import numpy as np

E = 8
D = 1024
H = 4096
LOSS_COEF = 0.01
EPS = 1e-10
NK = D // 128  # contraction chunks
NH = H // 512  # output column chunks


def _build(t_pad: int, with_bias: bool):
    from concourse import bacc, mybir
    import concourse.tile as tile

    mch = t_pad // 128
    # token groups of 384 (>=256 keeps fp32r matmul at full rate)
    tch = []
    s = 0
    while s < t_pad:
        tch.append((s, min(384, t_pad - s)))
        s += 384
    f32 = mybir.dt.float32
    f32r = mybir.dt.float32r
    AF = mybir.ActivationFunctionType
    mult = mybir.AluOpType.mult

    nc = bacc.Bacc(None)
    xt_d = nc.declare_dram_parameter("xt", [D, t_pad], f32r, isOutput=False)
    w_d = nc.declare_dram_parameter("w", [D, H], f32r, isOutput=False)
    wg_d = nc.declare_dram_parameter("wg", [D, E], f32r, isOutput=False)
    sel_d = nc.declare_dram_parameter("sel", [E, t_pad], f32, isOutput=False)
    pm_d = nc.declare_dram_parameter("pm", [1, t_pad], f32, isOutput=False)
    if with_bias:
        bb_d = nc.declare_dram_parameter("bb", [128, H], f32, isOutput=False)
    u_d = nc.declare_dram_parameter("u", [t_pad, H], f32, isOutput=True)
    imp_d = nc.declare_dram_parameter("imp", [1, 1], f32, isOutput=True)

    w_r = w_d.rearrange("(k p) h -> p k h", p=128)

    with tile.TileContext(nc) as tc:
        with (
            tc.tile_pool(name="res", bufs=1) as res_pool,
            tc.tile_pool(name="gat", bufs=1) as gat_pool,
            tc.tile_pool(name="wts", bufs=3) as w_pool,
            tc.tile_pool(name="out", bufs=6) as out_pool,
            tc.tile_pool(name="psm", bufs=5, space="PSUM") as psm,
            tc.tile_pool(name="psl", bufs=2, space="PSUM") as psl,
            tc.tile_pool(name="psd", bufs=1, space="PSUM") as psd,
            tc.tile_pool(name="dscr", bufs=1, space="DRAM") as dscr,
        ):
            # ---- static SBUF residents; triggers spread across the engines
            # whose preambles finish early (SyncE's takes ~7us) ----
            xt_sb = []
            for k in range(NK):
                t = res_pool.tile([128, t_pad], f32r, tag=f"xt{k}")
                eng = (nc.scalar, nc.gpsimd)[k % 2]
                eng.dma_start(t[:], xt_d[k * 128 : (k + 1) * 128, :])
                xt_sb.append(t)

            wg_sb = res_pool.tile([128, NK, E], f32r, tag="wg")
            nc.scalar.dma_start(wg_sb[:], wg_d.rearrange("(k p) e -> p k e", p=128))
            sel_sb = res_pool.tile([E, t_pad], f32, tag="sel")
            nc.scalar.dma_start(sel_sb[:], sel_d[:])
            pm_sb = res_pool.tile([1, t_pad], f32, tag="pm")
            nc.scalar.dma_start(pm_sb[:], pm_d[:])
            ones8 = res_pool.tile([E, 1], f32, tag="ones8")
            nc.vector.memset(ones8[:], 1.0)
            if with_bias:
                bb_sb = res_pool.tile([128, H], f32, tag="bb")
                nc.scalar.dma_start(bb_sb[:], bb_d[:])

            def load_w_block(n, eng):
                wt = w_pool.tile([128, NK, 512], f32r, tag="w")
                eng.dma_start(wt[:], w_r[:, :, n * 512 : (n + 1) * 512])
                return wt

            gcol = gat_pool.tile([128, mch], f32)  # per-token gate, token-major
            g_row = gat_pool.tile([1, t_pad], f32, tag="grow")
            gscr = dscr.tile([1, t_pad], f32)

            wt0 = load_w_block(0, nc.scalar)

            def emit_main(wt, n, m, trig):
                ps = psm.tile([128, 512], f32)
                for k in range(NK):
                    nc.tensor.matmul(
                        ps[:],
                        xt_sb[k][:, m * 128 : (m + 1) * 128],
                        wt[:, k, :],
                        start=(k == 0),
                        stop=(k == NK - 1),
                    )
                ot = out_pool.tile([128, 512], f32)
                nc.scalar.activation(ot[:], ps[:], AF.Copy, scale=gcol[:, m : m + 1])
                if with_bias:
                    nc.vector.scalar_tensor_tensor(
                        ot[:],
                        bb_sb[:, n * 512 : (n + 1) * 512],
                        gcol[:, m : m + 1],
                        ot[:],
                        op0=mult,
                        op1=mybir.AluOpType.add,
                    )
                trig.dma_start(
                    u_d[m * 128 : (m + 1) * 128, n * 512 : (n + 1) * 512], ot[:]
                )

            # ---- gates per token-group, pipelined with n=0 matmuls ----
            trigs = [nc.sync, nc.gpsimd]
            for g, (t0, tn) in enumerate(tch):
                # logitsT chunk [E, tn] = sum_k wg[k].T @ xt[k][:, chunk]
                lps = psl.tile([8, 512], f32)
                for k in range(NK):
                    nc.tensor.matmul(
                        lps[:E, :tn],
                        wg_sb[:, k, :],
                        xt_sb[k][:, t0 : t0 + tn],
                        start=(k == 0),
                        stop=(k == NK - 1),
                    )
                prod = gat_pool.tile([E, 512], f32, tag="prod")
                nc.vector.tensor_mul(
                    prod[:E, :tn], lps[:E, :tn], sel_sb[:, t0 : t0 + tn]
                )
                dps = psd.tile([1, 512], f32)
                nc.tensor.matmul(
                    dps[:1, :tn], ones8[:], prod[:E, :tn], start=True, stop=True
                )
                sig = gat_pool.tile([1, 512], f32, tag="sig")
                nc.scalar.activation(sig[:1, :tn], dps[:1, :tn], AF.Sigmoid)
                nc.vector.tensor_mul(
                    g_row[:, t0 : t0 + tn], sig[:1, :tn], pm_sb[:, t0 : t0 + tn]
                )
                # token-major gate columns for this group via a DRAM bounce
                # (an SBUF AP can't turn a free index into a partition index)
                nc.gpsimd.dma_start(gscr[:, t0 : t0 + tn], g_row[:, t0 : t0 + tn])
                nc.gpsimd.dma_start(
                    gcol[:, t0 // 128 : (t0 + tn) // 128],
                    gscr[:, t0 : t0 + tn].rearrange("a (m p) -> p (m a)", p=128),
                )
                # n=0 main matmuls for this group's token chunks
                for j, m in enumerate(range(t0 // 128, (t0 + tn) // 128)):
                    emit_main(wt0, 0, m, trigs[j % 2])

            imp_sb = gat_pool.tile([1, 1], f32, tag="imp")
            nc.vector.reduce_sum(imp_sb[:], g_row[:], axis=mybir.AxisListType.X)
            nc.gpsimd.dma_start(imp_d[:], imp_sb[:])

            # ---- remaining n-blocks ----
            for n in range(1, NH):
                wt = load_w_block(n, nc.sync)
                for m in range(mch):
                    emit_main(wt, n, m, trigs[m % 2])
    nc.compile()
    return nc


def _cv_squared(v: np.ndarray) -> np.float32:
    v = v.astype(np.float32)
    return np.float32(v.var(ddof=1) / (v.mean() ** 2 + EPS))


def kernel(x, w_gate, expert_w, expert_b):
    from concourse.bass_utils import run_bass_kernel_spmd

    x = np.asarray(x, dtype=np.float32)
    w_gate = np.asarray(w_gate, dtype=np.float32)
    expert_w = np.ascontiguousarray(np.asarray(expert_w, dtype=np.float32))
    expert_b = np.asarray(expert_b, dtype=np.float32)
    B, S, _ = x.shape
    N = B * S
    xf = x.reshape(N, D)

    # ---- host routing (sharding decision only; fp64 so the top-2 selection
    # matches the fp32 reference even through near-ties) ----
    logits = xf.astype(np.float64) @ w_gate.astype(np.float64)
    order = np.argsort(-logits, axis=1, kind="stable")
    top1, top2 = order[:, 0].copy(), order[:, 1].copy()

    idx = [np.nonzero((top1 == e) | (top2 == e))[0] for e in range(E)]
    counts = np.array([len(i) for i in idx], dtype=np.int64)
    t_pad = max(384, int(-(-counts.max() // 128) * 128))

    with_bias = bool(np.any(expert_b))
    nc = _build(t_pad, with_bias)

    in_maps = []
    for e in range(E):
        ids = idx[e]
        t = len(ids)
        xt = np.zeros((D, t_pad), np.float32)
        xt[:, :t] = xf[ids].T
        sel = np.zeros((E, t_pad), np.float32)
        rows = np.arange(t)
        sel[e, rows] = 1.0
        other = np.where(top1[ids] == e, top2[ids], top1[ids])
        sel[other, rows] -= 1.0
        pm = np.zeros((1, t_pad), np.float32)
        pm[0, :t] = 1.0
        m = {"xt": xt, "w": expert_w[e], "wg": w_gate, "sel": sel, "pm": pm}
        if with_bias:
            m["bb"] = np.broadcast_to(expert_b[e], (128, H)).copy()
        in_maps.append(m)

    res = run_bass_kernel_spmd(nc, in_maps, list(range(E)))
    kernel.last_results = res

    # ---- unshard: scatter-add the two expert contributions per token ----
    y = np.zeros((N, H), np.float32)
    imp = np.zeros(E, np.float32)
    for e in range(E):
        u = res.results[e]["u"]
        y[idx[e]] += u[: counts[e]]
        imp[e] = res.results[e]["imp"][0, 0]
    load = counts.astype(np.float32)
    loss = np.float32(LOSS_COEF) * (_cv_squared(imp) + _cv_squared(load))
    return y.reshape(B, S, H), np.float32(loss)
